# revision 31
# baseline (speedup 1.0000x reference)
"""Batched QK^T matmul on 8 Trainium2 NeuronCores.

Problem: mat_0 [8, 2048, 1024] f32, mat_1 [8, 2048, 1024] f32
         out   [8, 2048, 2048] f32 = einsum('bne,bme->bnm')

Sharding: data-parallel over batch — core i computes C = A @ B^T with
A = mat_0[i], B = mat_1[i].

Modes:
  f32   — exact: fp32 PE transposes + fp32 matmuls (4 cyc/row).
  f32r  — fp32 data, float32r matmuls (1 cyc/row, ~13-bit mantissa,
          rel err ~1.3e-4). PE transposes.
  fp16x / bf16x — host pre-casts inputs to fp16/bf16; on-chip XBAR
          DMA-transposes (no PE transpose work); 1 cyc/row matmuls.
  fp16s3 / bf16s3 — split precision: host ships hi and lo = x - hi;
          C ~= hi@hi + hi@lo + lo@hi (3 matmuls, near-fp32 accuracy).
"""

import sys

if "/opt/trn_rl_repo" not in sys.path:
    sys.path.insert(0, "/opt/trn_rl_repo")

import numpy as np

import concourse.mybir as mybir  # noqa: E402
import concourse.tile as tile  # noqa: E402
from concourse import bacc  # noqa: E402
from concourse.bass_utils import run_bass_kernel_spmd  # noqa: E402
from concourse.masks import make_identity  # noqa: E402

P = 128

# Hardcoded problem shape (nn_AttentionMatrix_41841571398230)
B_FULL, N_FULL, M_FULL, E_FULL = 8, 2048, 2048, 1024


# --------------------------------------------------------------------------
# PE-transpose path (f32 / f32r): full-precision operands
# --------------------------------------------------------------------------
def qkt_kernel_petp(tc, a, b, c, n, m, e, mm_mode, mg=512):
    nc = tc.nc
    f32 = mybir.dt.float32
    op_dtype = {"f32": f32, "f32r": mybir.dt.float32r}[mm_mode]
    mg = min(mg, m)
    n_blocks = n // P
    m_blocks = m // P
    e_chunks = e // P
    m_groups = m // mg

    with (
        tc.tile_pool(name="const", bufs=1) as const_pool,
        tc.tile_pool(name="stage", bufs=3) as stage_pool,
        tc.tile_pool(name="tpsum", bufs=2, space="PSUM") as tpsum_pool,
        tc.tile_pool(name="btp", bufs=1) as bt_pool,
        tc.tile_pool(name="atp", bufs=2) as at_pool,
        tc.tile_pool(name="mpsum", bufs=4, space="PSUM") as mpsum_pool,
        tc.tile_pool(name="co", bufs=3) as co_pool,
    ):
        ident = const_pool.tile([P, P], f32)
        make_identity(nc, ident)

        # B^T cached in SBUF: bt[p, k, m] = B[m, k*128+p]
        bt = bt_pool.tile([P, e_chunks, m], op_dtype)
        for mb in range(m_blocks):
            stage = stage_pool.tile([P, e], f32, tag="stage")
            nc.sync.dma_start(stage, b[mb * P : (mb + 1) * P, :])
            for k in range(e_chunks):
                pt = tpsum_pool.tile([P, P], f32, tag="tp")
                nc.tensor.transpose(pt, stage[:, k * P : (k + 1) * P], ident)
                nc.scalar.copy(bt[:, k, mb * P : (mb + 1) * P], pt)

        for nb in range(n_blocks):
            stage = stage_pool.tile([P, e], f32, tag="stage")
            nc.sync.dma_start(stage, a[nb * P : (nb + 1) * P, :])
            at = at_pool.tile([P, e_chunks, P], op_dtype, tag="at")
            for k in range(e_chunks):
                pt = tpsum_pool.tile([P, P], f32, tag="tp")
                nc.tensor.transpose(pt, stage[:, k * P : (k + 1) * P], ident)
                nc.scalar.copy(at[:, k, :], pt)

            for g in range(m_groups):
                ps = mpsum_pool.tile([P, mg], f32, tag="ps")
                for k in range(e_chunks):
                    nc.tensor.matmul(
                        ps,
                        at[:, k, :],
                        bt[:, k, g * mg : (g + 1) * mg],
                        start=(k == 0),
                        stop=(k == e_chunks - 1),
                    )
                ot = co_pool.tile([P, mg], f32, tag="ot")
                nc.vector.tensor_copy(ot, ps)
                nc.sync.dma_start(c[nb * P : (nb + 1) * P, g * mg : (g + 1) * mg], ot)


# --------------------------------------------------------------------------
# XBAR path (fp16x / bf16x / fp16s3 / bf16s3): host pre-cast 16-bit inputs
# --------------------------------------------------------------------------
def qkt_kernel_xbar(tc, ins, c, n, m, e, dt16, split, mg=512):
    """ins: (a_hi, b_hi) or (a_hi, a_lo, b_hi, b_lo) DRAM handles, dtype dt16.

    C = sum of term matmuls:
      split=False: C = a @ b^T
      split=True:  C = ahi@bhi^T + ahi@blo^T + alo@bhi^T

    Structure: one full-width XBAR DMA-transpose per (source, e-chunk)
    into per-chunk SBUF tiles (so dependencies release per chunk), then
    k-outer matmul emission over sets of 8 PSUM banks so the PE starts
    as soon as the first chunks land and stays busy during the serial
    XBAR phase.
    """
    nc = tc.nc
    f32 = mybir.dt.float32
    mg = min(mg, m)
    n_blocks = n // P
    e_chunks = e // P
    m_groups = m // mg

    if split:
        a_hi, a_lo, b_hi, b_lo = ins
        terms = [("ah", "bh"), ("ah", "bl"), ("al", "bh")]
        srcs = {"ah": a_hi, "al": a_lo, "bh": b_hi, "bl": b_lo}
    else:
        a_hi, b_hi = ins
        terms = [("ah", "bh")]
        srcs = {"ah": a_hi, "bh": b_hi}

    pe_tp = False  # PE-offloaded k7 transposes measured +70us (in-order PE stalls behind staged loads)
    with (
        tc.tile_pool(name="tpt", bufs=1) as tp_pool,
        tc.tile_pool(name="stg", bufs=4) as stg_pool,
        tc.tile_pool(name="tps", bufs=1, space="PSUM") as tps_pool,
        tc.tile_pool(name="mpsum", bufs=7 if pe_tp else 8, space="PSUM") as mpsum_pool,
        tc.tile_pool(name="co", bufs=4) as co_pool,
    ):
        # Full-width transposed chunk tiles: tag -> [k] -> [P, rows] fp16.
        # The XBAR is a serial ~233 GB/s pipe: total transpose time scales
        # with bytes, not op count, so per-chunk ops (earliest first-chunk
        # arrival) beat merged ops.
        n_xbar = e_chunks - 1 if pe_tp else e_chunks
        tchunks = {tag: [None] * e_chunks for tag in srcs}
        for k in range(n_xbar):
            for tag, src in srcs.items():
                rows = n if tag[0] == "a" else m
                t = tp_pool.tile([P, rows], dt16, name=f"t_{tag}{k}")
                if k == 0:
                    # first chunk in row-halves: set 0 only needs the first
                    # half of each, so its k=0 matmuls start ~2 us earlier
                    h = rows // 2
                    nc.sync.dma_start_transpose(t[:, :h], src[:h, 0:P])
                    nc.sync.dma_start_transpose(t[:, h:], src[h:, 0:P])
                else:
                    nc.sync.dma_start_transpose(t, src[:, k * P : (k + 1) * P])
                tchunks[tag][k] = t
        if pe_tp:
            # k = e_chunks-1 via PE identity transposes fed by SWDGE loads
            # (runs in the PE's XBAR-starvation window, parallel DMA path)
            ident = tp_pool.tile([P, P], dt16, name="ident16")
            make_identity(nc, ident)
            kl = e_chunks - 1
            for tag, src in srcs.items():
                rows = n if tag[0] == "a" else m
                t = tp_pool.tile([P, rows], dt16, name=f"t_{tag}{kl}")
                tchunks[tag][kl] = t
                for rb in range(rows // P):
                    stg = stg_pool.tile([P, P], dt16, tag="stg")
                    nc.gpsimd.dma_start(
                        stg, src[rb * P : (rb + 1) * P, kl * P : (kl + 1) * P]
                    )
                    pt = tps_pool.tile([P, P], dt16, tag="tp")
                    nc.tensor.transpose(pt, stg, ident)
                    nc.scalar.copy(t[:, rb * P : (rb + 1) * P], pt)

        # units = (g, nb) output tiles, processed in sets of 8 PSUM banks.
        # The first set has 7 units: its 8th bank runs warm-up filler
        # matmuls in the XBAR-starvation gaps so the HAM clock gate stays
        # at full rate (otherwise ~58 early matmuls run at 1.2 GHz).
        units = [(g, nb) for g in range(m_groups) for nb in range(n_blocks)]
        n_acc = len(terms) * e_chunks
        setsz = 7 if pe_tp else 8
        sets = [units[i0 : i0 + setsz] for i0 in range(0, len(units), setsz)]
        for si, chunk_units in enumerate(sets):
            pss = [
                mpsum_pool.tile([P, mg], f32, tag="ps", name=f"ps_{si}_{u}")
                for u in range(len(chunk_units))
            ]
            i = 0
            for ta, tb in terms:
                for k in range(e_chunks):
                    for u, (g, nb) in enumerate(chunk_units):
                        nc.tensor.matmul(
                            pss[u],
                            tchunks[ta][k][:, nb * P : (nb + 1) * P],
                            tchunks[tb][k][:, g * mg : (g + 1) * mg],
                            start=(i == 0),
                            stop=(i == n_acc - 1),
                        )
                    i += 1
            for u, (g, nb) in enumerate(chunk_units):
                ot = co_pool.tile([P, mg], f32, tag="ot")
                nc.vector.tensor_copy(ot, pss[u])
                nc.scalar.dma_start(
                    c[nb * P : (nb + 1) * P, g * mg : (g + 1) * mg], ot
                )


# --------------------------------------------------------------------------
# Builders
# --------------------------------------------------------------------------
def build_qkt(n, m, e, mm_mode="f32r", mg=512):
    f32 = mybir.dt.float32
    nc = bacc.Bacc(None, target_bir_lowering=False)
    xbar = mm_mode in ("fp16x", "bf16x", "fp16s3", "bf16s3")
    dt16 = mybir.dt.float16 if mm_mode.startswith("fp16") else mybir.dt.bfloat16
    split = mm_mode.endswith("s3")
    with tile.TileContext(nc) as tc:
        with tc.tile_pool(name="dram", bufs=1, space="DRAM") as dram:
            c = dram.tile([n, m], f32, kind="ExternalOutput", name="out")
            if not xbar:
                a = dram.tile([n, e], f32, kind="ExternalInput", name="mat_0")
                b = dram.tile([m, e], f32, kind="ExternalInput", name="mat_1")
                qkt_kernel_petp(tc, a[:], b[:], c[:], n, m, e, mm_mode, mg=mg)
                in_names = [a.name, b.name]
            else:
                names = ["a_hi", "a_lo", "b_hi", "b_lo"] if split else ["a_hi", "b_hi"]
                handles = []
                for nm_ in names:
                    rows = n if nm_.startswith("a") else m
                    handles.append(
                        dram.tile([rows, e], dt16, kind="ExternalInput", name=nm_)
                    )
                qkt_kernel_xbar(
                    tc, [h[:] for h in handles], c[:], n, m, e, dt16, split, mg=mg
                )
                in_names = [h.name for h in handles]
    nc.compile()
    return nc, in_names, c.name


_CACHE = {}


def _get_built(n, m, e, mm_mode, mg=512):
    key = (n, m, e, mm_mode, mg)
    if key not in _CACHE:
        _CACHE[key] = build_qkt(n, m, e, mm_mode=mm_mode, mg=mg)
    return _CACHE[key]


def _np16(mm_mode):
    import ml_dtypes

    return np.float16 if mm_mode.startswith("fp16") else ml_dtypes.bfloat16


def prep_inputs(mat_0, mat_1, mm_mode, in_names):
    """Host-side per-core input prep for each mode."""
    bsz = mat_0.shape[0]
    if mm_mode in ("f32", "f32r"):
        return [
            {
                in_names[0]: np.ascontiguousarray(mat_0[i], dtype=np.float32),
                in_names[1]: np.ascontiguousarray(mat_1[i], dtype=np.float32),
            }
            for i in range(bsz)
        ]
    t16 = _np16(mm_mode)

    if mm_mode in ("fp16x", "bf16x"):
        a16 = mat_0.astype(t16)
        b16 = mat_1.astype(t16)
        return [{in_names[0]: a16[i], in_names[1]: b16[i]} for i in range(bsz)]
    # split modes
    a_hi = mat_0.astype(t16)
    a_lo = (mat_0 - a_hi.astype(np.float32)).astype(t16)
    b_hi = mat_1.astype(t16)
    b_lo = (mat_1 - b_hi.astype(np.float32)).astype(t16)
    arrs = [a_hi, a_lo, b_hi, b_lo]
    return [
        {nm_: arrs[j][i] for j, nm_ in enumerate(in_names)} for i in range(bsz)
    ]


def run_qkt(mat_0, mat_1, mm_mode="f32r", mg=512, trace=False):
    """Run the sharded kernel on full inputs [b, n, e], [b, m, e]."""
    bsz, n, e = mat_0.shape
    _, m, _ = mat_1.shape
    nc, in_names, c_name = _get_built(n, m, e, mm_mode, mg)
    in_maps = prep_inputs(mat_0, mat_1, mm_mode, in_names)
    res = run_bass_kernel_spmd(nc, in_maps, core_ids=list(range(bsz)), trace=trace)
    out = np.stack([res.results[i][c_name] for i in range(bsz)], axis=0)
    return out, res


DEFAULT_MODE = "fp16x"


def kernel(mat_0, mat_1):
    out, _ = run_qkt(
        np.asarray(mat_0, dtype=np.float32),
        np.asarray(mat_1, dtype=np.float32),
        mm_mode=DEFAULT_MODE,
    )
    return out


# revision 32
# speedup vs baseline: 1.0564x; 1.0564x over previous
"""Batched QK^T matmul on 8 Trainium2 NeuronCores.

Problem: mat_0 [8, 2048, 1024] f32, mat_1 [8, 2048, 1024] f32
         out   [8, 2048, 2048] f32 = einsum('bne,bme->bnm')

Sharding: data-parallel over batch — core i computes C = A @ B^T with
A = mat_0[i], B = mat_1[i].

Modes:
  f32   — exact: fp32 PE transposes + fp32 matmuls (4 cyc/row).
  f32r  — fp32 data, float32r matmuls (1 cyc/row, ~13-bit mantissa,
          rel err ~1.3e-4). PE transposes.
  fp16x / bf16x — host pre-casts inputs to fp16/bf16; on-chip XBAR
          DMA-transposes (no PE transpose work); 1 cyc/row matmuls.
  fp16s3 / bf16s3 — split precision: host ships hi and lo = x - hi;
          C ~= hi@hi + hi@lo + lo@hi (3 matmuls, near-fp32 accuracy).
"""

import sys

if "/opt/trn_rl_repo" not in sys.path:
    sys.path.insert(0, "/opt/trn_rl_repo")

import numpy as np

import concourse.mybir as mybir  # noqa: E402
import concourse.tile as tile  # noqa: E402
from concourse import bacc  # noqa: E402
from concourse.bass_utils import run_bass_kernel_spmd  # noqa: E402
from concourse.masks import make_identity  # noqa: E402

P = 128

# Hardcoded problem shape (nn_AttentionMatrix_41841571398230)
B_FULL, N_FULL, M_FULL, E_FULL = 8, 2048, 2048, 1024


# --------------------------------------------------------------------------
# PE-transpose path (f32 / f32r): full-precision operands
# --------------------------------------------------------------------------
def qkt_kernel_petp(tc, a, b, c, n, m, e, mm_mode, mg=512):
    nc = tc.nc
    f32 = mybir.dt.float32
    op_dtype = {"f32": f32, "f32r": mybir.dt.float32r}[mm_mode]
    mg = min(mg, m)
    n_blocks = n // P
    m_blocks = m // P
    e_chunks = e // P
    m_groups = m // mg

    with (
        tc.tile_pool(name="const", bufs=1) as const_pool,
        tc.tile_pool(name="stage", bufs=3) as stage_pool,
        tc.tile_pool(name="tpsum", bufs=2, space="PSUM") as tpsum_pool,
        tc.tile_pool(name="btp", bufs=1) as bt_pool,
        tc.tile_pool(name="atp", bufs=2) as at_pool,
        tc.tile_pool(name="mpsum", bufs=4, space="PSUM") as mpsum_pool,
        tc.tile_pool(name="co", bufs=3) as co_pool,
    ):
        ident = const_pool.tile([P, P], f32)
        make_identity(nc, ident)

        # B^T cached in SBUF: bt[p, k, m] = B[m, k*128+p]
        bt = bt_pool.tile([P, e_chunks, m], op_dtype)
        for mb in range(m_blocks):
            stage = stage_pool.tile([P, e], f32, tag="stage")
            nc.sync.dma_start(stage, b[mb * P : (mb + 1) * P, :])
            for k in range(e_chunks):
                pt = tpsum_pool.tile([P, P], f32, tag="tp")
                nc.tensor.transpose(pt, stage[:, k * P : (k + 1) * P], ident)
                nc.scalar.copy(bt[:, k, mb * P : (mb + 1) * P], pt)

        for nb in range(n_blocks):
            stage = stage_pool.tile([P, e], f32, tag="stage")
            nc.sync.dma_start(stage, a[nb * P : (nb + 1) * P, :])
            at = at_pool.tile([P, e_chunks, P], op_dtype, tag="at")
            for k in range(e_chunks):
                pt = tpsum_pool.tile([P, P], f32, tag="tp")
                nc.tensor.transpose(pt, stage[:, k * P : (k + 1) * P], ident)
                nc.scalar.copy(at[:, k, :], pt)

            for g in range(m_groups):
                ps = mpsum_pool.tile([P, mg], f32, tag="ps")
                for k in range(e_chunks):
                    nc.tensor.matmul(
                        ps,
                        at[:, k, :],
                        bt[:, k, g * mg : (g + 1) * mg],
                        start=(k == 0),
                        stop=(k == e_chunks - 1),
                    )
                ot = co_pool.tile([P, mg], f32, tag="ot")
                nc.vector.tensor_copy(ot, ps)
                nc.sync.dma_start(c[nb * P : (nb + 1) * P, g * mg : (g + 1) * mg], ot)


# --------------------------------------------------------------------------
# XBAR path (fp16x / bf16x / fp16s3 / bf16s3): host pre-cast 16-bit inputs
# --------------------------------------------------------------------------
def qkt_kernel_xbar(tc, ins, c, n, m, e, dt16, split, mg=512):
    """ins: (a_hi, b_hi) or (a_hi, a_lo, b_hi, b_lo) DRAM handles, dtype dt16.

    C = sum of term matmuls:
      split=False: C = a @ b^T
      split=True:  C = ahi@bhi^T + ahi@blo^T + alo@bhi^T

    Structure: one full-width XBAR DMA-transpose per (source, e-chunk)
    into per-chunk SBUF tiles (so dependencies release per chunk), then
    k-outer matmul emission over sets of 8 PSUM banks so the PE starts
    as soon as the first chunks land and stays busy during the serial
    XBAR phase.
    """
    nc = tc.nc
    f32 = mybir.dt.float32
    mg = min(mg, m)
    n_blocks = n // P
    e_chunks = e // P
    m_groups = m // mg

    if split:
        a_hi, a_lo, b_hi, b_lo = ins
        terms = [("ah", "bh"), ("ah", "bl"), ("al", "bh")]
        srcs = {"ah": a_hi, "al": a_lo, "bh": b_hi, "bl": b_lo}
    else:
        a_hi, b_hi = ins
        terms = [("ah", "bh")]
        srcs = {"ah": a_hi, "bh": b_hi}

    pe_tp = False  # PE-offloaded k7 transposes measured +70us (in-order PE stalls behind staged loads)
    with (
        tc.tile_pool(name="tpt", bufs=1) as tp_pool,
        tc.tile_pool(name="stg", bufs=4) as stg_pool,
        tc.tile_pool(name="tps", bufs=1, space="PSUM") as tps_pool,
        tc.tile_pool(name="mpsum", bufs=7 if pe_tp else 8, space="PSUM") as mpsum_pool,
        tc.tile_pool(name="co", bufs=4) as co_pool,
    ):
        # Full-width transposed chunk tiles: tag -> [k] -> [P, rows] fp16.
        # The XBAR is a serial ~233 GB/s pipe: total transpose time scales
        # with bytes, not op count, so per-chunk ops (earliest first-chunk
        # arrival) beat merged ops.
        n_xbar = e_chunks - 1 if pe_tp else e_chunks
        tchunks = {tag: [None] * e_chunks for tag in srcs}
        for k in range(n_xbar):
            for tag, src in srcs.items():
                rows = n if tag[0] == "a" else m
                t = tp_pool.tile([P, rows], dt16, name=f"t_{tag}{k}")
                nc.sync.dma_start_transpose(t, src[:, k * P : (k + 1) * P])
                tchunks[tag][k] = t
        if pe_tp:
            # k = e_chunks-1 via PE identity transposes fed by SWDGE loads
            # (runs in the PE's XBAR-starvation window, parallel DMA path)
            ident = tp_pool.tile([P, P], dt16, name="ident16")
            make_identity(nc, ident)
            kl = e_chunks - 1
            for tag, src in srcs.items():
                rows = n if tag[0] == "a" else m
                t = tp_pool.tile([P, rows], dt16, name=f"t_{tag}{kl}")
                tchunks[tag][kl] = t
                for rb in range(rows // P):
                    stg = stg_pool.tile([P, P], dt16, tag="stg")
                    nc.gpsimd.dma_start(
                        stg, src[rb * P : (rb + 1) * P, kl * P : (kl + 1) * P]
                    )
                    pt = tps_pool.tile([P, P], dt16, tag="tp")
                    nc.tensor.transpose(pt, stg, ident)
                    nc.scalar.copy(t[:, rb * P : (rb + 1) * P], pt)

        # units = (g, nb) output tiles, processed in sets of 8 PSUM banks.
        # The first set has 7 units: its 8th bank runs warm-up filler
        # matmuls in the XBAR-starvation gaps so the HAM clock gate stays
        # at full rate (otherwise ~58 early matmuls run at 1.2 GHz).
        units = [(g, nb) for g in range(m_groups) for nb in range(n_blocks)]
        n_acc = len(terms) * e_chunks
        setsz = 7 if pe_tp else 8
        sets = [units[i0 : i0 + setsz] for i0 in range(0, len(units), setsz)]
        for si, chunk_units in enumerate(sets):
            pss = [
                mpsum_pool.tile([P, mg], f32, tag="ps", name=f"ps_{si}_{u}")
                for u in range(len(chunk_units))
            ]
            i = 0
            for ta, tb in terms:
                for k in range(e_chunks):
                    for u, (g, nb) in enumerate(chunk_units):
                        nc.tensor.matmul(
                            pss[u],
                            tchunks[ta][k][:, nb * P : (nb + 1) * P],
                            tchunks[tb][k][:, g * mg : (g + 1) * mg],
                            start=(i == 0),
                            stop=(i == n_acc - 1),
                        )
                    i += 1
            for u, (g, nb) in enumerate(chunk_units):
                ot = co_pool.tile([P, mg], f32, tag="ot")
                nc.vector.tensor_copy(ot, pss[u])
                nc.scalar.dma_start(
                    c[nb * P : (nb + 1) * P, g * mg : (g + 1) * mg], ot
                )


# --------------------------------------------------------------------------
# Builders
# --------------------------------------------------------------------------
def build_qkt(n, m, e, mm_mode="f32r", mg=512):
    f32 = mybir.dt.float32
    nc = bacc.Bacc(None, target_bir_lowering=False)
    xbar = mm_mode in ("fp16x", "bf16x", "fp16s3", "bf16s3")
    dt16 = mybir.dt.float16 if mm_mode.startswith("fp16") else mybir.dt.bfloat16
    split = mm_mode.endswith("s3")
    with tile.TileContext(nc) as tc:
        with tc.tile_pool(name="dram", bufs=1, space="DRAM") as dram:
            c = dram.tile([n, m], f32, kind="ExternalOutput", name="out")
            if not xbar:
                a = dram.tile([n, e], f32, kind="ExternalInput", name="mat_0")
                b = dram.tile([m, e], f32, kind="ExternalInput", name="mat_1")
                qkt_kernel_petp(tc, a[:], b[:], c[:], n, m, e, mm_mode, mg=mg)
                in_names = [a.name, b.name]
            else:
                names = ["a_hi", "a_lo", "b_hi", "b_lo"] if split else ["a_hi", "b_hi"]
                handles = []
                for nm_ in names:
                    rows = n if nm_.startswith("a") else m
                    handles.append(
                        dram.tile([rows, e], dt16, kind="ExternalInput", name=nm_)
                    )
                qkt_kernel_xbar(
                    tc, [h[:] for h in handles], c[:], n, m, e, dt16, split, mg=mg
                )
                in_names = [h.name for h in handles]
    nc.compile()
    return nc, in_names, c.name


_CACHE = {}


def _get_built(n, m, e, mm_mode, mg=512):
    key = (n, m, e, mm_mode, mg)
    if key not in _CACHE:
        _CACHE[key] = build_qkt(n, m, e, mm_mode=mm_mode, mg=mg)
    return _CACHE[key]


def _np16(mm_mode):
    import ml_dtypes

    return np.float16 if mm_mode.startswith("fp16") else ml_dtypes.bfloat16


def prep_inputs(mat_0, mat_1, mm_mode, in_names):
    """Host-side per-core input prep for each mode."""
    bsz = mat_0.shape[0]
    if mm_mode in ("f32", "f32r"):
        return [
            {
                in_names[0]: np.ascontiguousarray(mat_0[i], dtype=np.float32),
                in_names[1]: np.ascontiguousarray(mat_1[i], dtype=np.float32),
            }
            for i in range(bsz)
        ]
    t16 = _np16(mm_mode)

    if mm_mode in ("fp16x", "bf16x"):
        a16 = mat_0.astype(t16)
        b16 = mat_1.astype(t16)
        return [{in_names[0]: a16[i], in_names[1]: b16[i]} for i in range(bsz)]
    # split modes
    a_hi = mat_0.astype(t16)
    a_lo = (mat_0 - a_hi.astype(np.float32)).astype(t16)
    b_hi = mat_1.astype(t16)
    b_lo = (mat_1 - b_hi.astype(np.float32)).astype(t16)
    arrs = [a_hi, a_lo, b_hi, b_lo]
    return [
        {nm_: arrs[j][i] for j, nm_ in enumerate(in_names)} for i in range(bsz)
    ]


def run_qkt(mat_0, mat_1, mm_mode="f32r", mg=512, trace=False):
    """Run the sharded kernel on full inputs [b, n, e], [b, m, e]."""
    bsz, n, e = mat_0.shape
    _, m, _ = mat_1.shape
    nc, in_names, c_name = _get_built(n, m, e, mm_mode, mg)
    in_maps = prep_inputs(mat_0, mat_1, mm_mode, in_names)
    res = run_bass_kernel_spmd(nc, in_maps, core_ids=list(range(bsz)), trace=trace)
    out = np.stack([res.results[i][c_name] for i in range(bsz)], axis=0)
    return out, res


DEFAULT_MODE = "fp16x"


def kernel(mat_0, mat_1):
    out, _ = run_qkt(
        np.asarray(mat_0, dtype=np.float32),
        np.asarray(mat_1, dtype=np.float32),
        mm_mode=DEFAULT_MODE,
    )
    return out
